# revision 12
# baseline (speedup 1.0000x reference)
"""Distributed causal multi-head attention for one TRN2 chip (8 NeuronCores).

Sharding: 2 batches x 4 head-groups. Core c handles batch c//4 and heads
[4*(c%4), 4*(c%4)+4). Wq/Wk/Wv are column-sliced per head group, Wo is
row-sliced; each core emits a partial (S, D) output and the host sums the
4 partials per batch and adds bo. No on-device collectives.

Per-core dataflow (all matmuls contract over the SBUF partition dim):
  inputs are fed pre-transposed (D, S) so that
    Q^T, K^T = Wq/Wk-chunk.T @ X^T        (heads on partitions)
    V        = X^T-chunk.T  @ Wv          (tokens on partitions, + ones col)
    S^T      = K_h^T-chunk.T @ Q_h^T      (keys on partitions)
    P^T      = exp(scale * S^T)           (causal: restricted N + affine_select)
    O^T      = [V_h | 1].T @ P^T          (row 64 = softmax denominators)
    out      = O^T-chunk.T  @ Wo          (natural layout, DMA out)
"""

import math
import os
import sys

import ml_dtypes
import numpy as np

for _p in ("/opt/trn_rl_repo", "/root/.axon_site/_ro/trn_rl_repo"):
    if os.path.isdir(_p) and _p not in sys.path:
        sys.path.insert(0, _p)

import concourse.bass as bass
import concourse.mybir as mybir
import concourse.tile as tile
from concourse import bacc
from concourse.bass_utils import run_bass_kernel_spmd

F32 = mybir.dt.float32
F32R = mybir.dt.float32r
BF16 = mybir.dt.bfloat16
FP8 = mybir.dt.float8e4
NPBF16 = ml_dtypes.bfloat16
NPFP8 = ml_dtypes.float8_e4m3
AF = mybir.ActivationFunctionType
ALU = mybir.AluOpType


def _r(ap):
    # float32r: same bits as f32, streams the PE at 1 col/cycle (vs 4 for f32)
    return ap.bitcast(F32R)

B, S, D = 2, 2048, 1024
H, HD = 16, 64
NCORES, NGROUPS = 8, 4
GC = D // NGROUPS            # 256 attention columns per core (4 heads)
GH = GC // HD                # heads per core
SCALE = 1.0 / math.sqrt(S)   # reference divides by sqrt(Sk), not sqrt(hd)
WSC = 8.0                    # fp8 pre-scale on Wq/Wk (and bq/bk)
SCALE_QK = SCALE / (WSC * WSC)
DCH = D // 128               # contraction chunks over model dim
ST = S // 128                # 128-row token tiles
NJ = S // 512                # 512-wide query tiles


def _emit(tc, xq, xk, xv, wq, wk, wv, bq, bk, bv, wo, out):
    nc = tc.nc
    from contextlib import ExitStack

    with ExitStack() as ctx:
        const = ctx.enter_context(tc.tile_pool(name="const", bufs=1))
        perm = ctx.enter_context(tc.tile_pool(name="perm", bufs=1))
        xqk = ctx.enter_context(tc.tile_pool(name="xqk", bufs=4))
        xvp = ctx.enter_context(tc.tile_pool(name="xvp", bufs=2))
        ptp = ctx.enter_context(tc.tile_pool(name="ptp", bufs=5))
        nrm = ctx.enter_context(tc.tile_pool(name="nrm", bufs=4))
        outp = ctx.enter_context(tc.tile_pool(name="outp", bufs=1))
        pp = ctx.enter_context(tc.tile_pool(name="pp", bufs=2, space="PSUM"))
        sp = ctx.enter_context(tc.tile_pool(name="sp", bufs=2, space="PSUM"))
        op = ctx.enter_context(tc.tile_pool(name="op", bufs=2, space="PSUM"))

        def load_w(dram, tag):
            # dram is a host-preformatted SBUF image [128, nch, n]
            t = const.tile(list(dram.shape), dram.dtype, tag=tag, name=tag)
            nc.sync.dma_start(out=t[:], in_=dram[:, :, :])
            return t

        def load_b(dram, tag):
            t = const.tile([128, 2], F32, tag=tag, name=tag)
            nc.sync.dma_start(out=t[:], in_=dram.ap().rearrange("(m p) -> p m", p=128))
            return t

        # persistent activation tiles --------------------------------------
        qt = [[perm.tile([128, 512], BF16, tag=f"qt{m}_{jj}", name=f"qt{m}_{jj}")
               for jj in range(NJ)] for m in range(2)]
        kt = [[perm.tile([128, 512], BF16, tag=f"kt{m}_{jj}", name=f"kt{m}_{jj}")
               for jj in range(NJ)] for m in range(2)]
        ot = [[perm.tile([128, 512], BF16, tag=f"ot{c}_{j}", name=f"ot{c}_{j}")
               for j in range(NJ)] for c in range(2)]
        ones_f = const.tile([128, GH], BF16, tag="onesf", name="ones_f")
        nc.vector.memset(ones_f[:], 1.0)
        vaug = []
        for t in range(ST):
            vt = perm.tile([128, GH, HD + 1], BF16, tag=f"vaug{t}", name=f"vaug{t}")
            nc.vector.tensor_copy(out=vt[:, :, HD:HD + 1], in_=ones_f[:, :, None])
            vaug.append(vt)
        ones_row = const.tile([1, 128], BF16, tag="ones", name="ones")
        nc.vector.tensor_copy(out=ones_row[:], in_=ones_f[0:1, 0:1].broadcast_to([1, 128]))

        def load_half(x_dram, ha, pool, dt):
            # x_dram is a host-preformatted image [128, 2, DCH, 1024]
            xt = pool.tile([128, DCH, 1024], dt, tag=f"x{ha}", name="xct")
            nc.sync.dma_start(out=xt[:], in_=x_dram[:, ha, :, :])
            return xt

        def proj_qk_m(ha, xt, w_sb, b_sb, dst, m, act_bias=False):
            ps = [pp.tile([128, 512], F32, tag="pp", name="pqk") for _ in range(2)]
            for d in range(DCH):
                for jj in range(2):
                    nc.tensor.matmul(
                        ps[jj][:],
                        w_sb[:, d, 128 * m:128 * (m + 1)],
                        xt[:, d, 512 * jj:512 * (jj + 1)],
                        start=(d == 0), stop=(d == DCH - 1))
            for jj in range(2):
                if act_bias and jj == 1:
                    # scalar engine is idle here; halves PSUM-evacuation latency
                    nc.scalar.activation(
                        out=dst[m][2 * ha + jj][:], in_=ps[jj][:],
                        func=AF.Identity, bias=b_sb[:, m:m + 1])
                else:
                    nc.vector.tensor_scalar_add(
                        out=dst[m][2 * ha + jj][:], in0=ps[jj][:],
                        scalar1=b_sb[:, m:m + 1])

        def proj_qk(ha, xt, w_sb, b_sb, dst, act_bias=False):
            for m in range(2):
                proj_qk_m(ha, xt, w_sb, b_sb, dst, m, act_bias)

        def proj_v(ha, xt, wv_sb, bv_sb, tts):
            for tt in tts:
                t = 8 * ha + tt
                ps = pp.tile([128, GC], F32, tag="pp", name="pv")
                for d in range(DCH):
                    nc.tensor.matmul(
                        ps[:],
                        xt[:, d, 128 * tt:128 * (tt + 1)],
                        wv_sb[:, d, :],
                        start=(d == 0), stop=False)
                nc.tensor.matmul(
                    ps[:], ones_row[:], bv_sb[:], start=False, stop=True)
                nc.vector.tensor_copy(
                    out=vaug[t][:, :, 0:HD],
                    in_=ps[:].rearrange("p (h e) -> p h e", h=GH))

        def attn_j(j, fillers=()):
            """Pipelined attention for all GH heads of query tile j.

            Emits units (h, chunk-pair): S -> exp -> [mask]; PV trails LAG
            units behind so the PE never sits on the exp latency. `fillers`
            are zero-arg emitters (outproj pieces) spread between units to
            soak up PE idle while the scalar engine chews exp.
            """
            nch = 4 * (j + 1)
            ncp = nch // 2
            units = [(h, cp) for h in range(GH) for cp in range(ncp)]
            LAG = 3
            fillers = list(fillers)
            nf = len(fillers)
            nu = len(units)
            otps = {}
            pend = []

            def emit_S(h, cp):
                c, rr = divmod(h, 2)
                rr *= HD
                sp2 = sp.tile([128, 1024], F32, tag="sp2", name="sp2")
                offs = []
                for k2 in range(2):
                    i = 2 * cp + k2
                    o = i - 4 * j
                    off = min(128 * o, 256) if o > 0 else 0
                    offs.append((i, o, off))
                    nc.tensor.matmul(
                        sp2[:, 512 * k2 + off:512 * (k2 + 1)],
                        kt[c][i // 4][rr:rr + HD, 128 * (i % 4):128 * (i % 4) + 128],
                        qt[c][j][rr:rr + HD, off:512],
                        start=True, stop=True)
                pt2 = ptp.tile([128, 1024], BF16, tag="pt", name="ptt")
                off0 = offs[0][2]
                nc.scalar.activation(
                    out=pt2[:, off0:1024], in_=sp2[:, off0:1024],
                    func=AF.Exp, scale=SCALE_QK)
                for k2, (i, o, off) in enumerate(offs):
                    if o >= 0:
                        w = 256 if o == 3 else 128
                        nc.gpsimd.affine_select(
                            out=pt2[:, 512 * k2 + off:512 * k2 + off + w],
                            in_=pt2[:, 512 * k2 + off:512 * k2 + off + w],
                            compare_op=ALU.is_ge, fill=0.0, base=off - 128 * o,
                            pattern=[[1, w]], channel_multiplier=-1)
                return pt2, offs

            def emit_PV(h, cp, pt2, offs):
                for k2, (i, o, off) in enumerate(offs):
                    nc.tensor.matmul(
                        otps[h][:, off:512], vaug[i][:, h, :],
                        pt2[:, 512 * k2 + off:512 * (k2 + 1)],
                        start=(i == 0), stop=(i == nch - 1),
                        skip_group_check=True)

            def norm(h):
                c, rr = divmod(h, 2)
                rr *= HD
                otp = otps[h]
                rc = nrm.tile([1, 512], F32, tag="rc", name="rc")
                nc.vector.reciprocal(rc[:], otp[HD:HD + 1, :])
                bc = nrm.tile([HD, 512], F32, tag="bc", name="bc")
                nc.gpsimd.partition_broadcast(bc[:], rc[:])
                nc.vector.tensor_tensor(
                    out=ot[c][j][rr:rr + HD, :], in0=otp[0:HD, :], in1=bc[:],
                    op=ALU.mult)

            nxt_fill = 0

            def drain_one():
                h2, cp2, pt2, offs2 = pend.pop(0)
                emit_PV(h2, cp2, pt2, offs2)
                if cp2 == ncp - 1:
                    norm(h2)

            for idx, (h, cp) in enumerate(units):
                if cp == 0:
                    otps[h] = op.tile([HD + 1, 512], F32, tag="op", name="otp")
                pend.append((h, cp) + emit_S(h, cp))
                if len(pend) > LAG:
                    drain_one()
                while nf and nxt_fill < (idx + 1) * nf // nu:
                    fillers[nxt_fill]()
                    nxt_fill += 1
            while pend:
                drain_one()
            while nxt_fill < nf:
                fillers[nxt_fill]()
                nxt_fill += 1

        obs = {}

        def outproj_piece(j, wo_sb, tt):
            if tt == 0:
                obs[j] = outp.tile([128, 4, D], BF16, tag="ob", name="ob")
            ob = obs[j]
            for n2 in range(2):
                fpt = pp.tile([128, 512], F32, tag="pp", name="fpt")
                for c in range(2):
                    nc.tensor.matmul(
                        fpt[:],
                        ot[c][j][:, 128 * tt:128 * (tt + 1)],
                        wo_sb[:, c, 512 * n2:512 * (n2 + 1)],
                        start=(c == 0), stop=(c == 1))
                if n2 == 0:
                    nc.vector.tensor_copy(
                        out=ob[:, tt, 512 * n2:512 * (n2 + 1)], in_=fpt[:])
                else:
                    nc.scalar.copy(out=ob[:, tt, 512 * n2:512 * (n2 + 1)], in_=fpt[:])
            if tt == 3:
                nc.sync.dma_start(
                    out=out[512 * j:512 * (j + 1), :].rearrange(
                        "(c p) n -> p c n", p=128),
                    in_=ob[:])

        def op_pieces(j, wo_sb, tts=range(4)):
            return [(lambda j=j, tt=tt: outproj_piece(j, wo_sb, tt))
                    for tt in tts]

        # ---- emission schedule --------------------------------------------
        # Loads are emitted just-in-time (a consumer waits on the DMA-queue
        # counter, so hoisting unrelated loads ahead of it delays its start);
        # PE-heavy projection/outproj pieces are spread as fillers inside the
        # ACT(exp)-bound attention stretches so no engine starves.
        wk_sb = load_w(wk, "wk"); bk_sb = load_b(bk, "bk")
        xk0 = load_half(xk, 0, xqk, FP8)
        proj_qk(0, xk0, wk_sb, bk_sb, kt, act_bias=True)
        wq_sb = load_w(wq, "wq"); bq_sb = load_b(bq, "bq")
        xq0 = load_half(xq, 0, xqk, FP8)
        proj_qk(0, xq0, wq_sb, bq_sb, qt, act_bias=True)
        wv_sb = load_w(wv, "wv")
        bv_sb = const.tile([1, GC], BF16, tag="bv", name="bv")
        nc.sync.dma_start(out=bv_sb[:], in_=bv.ap().rearrange("(o n) -> o n", o=1))
        xv0 = load_half(xv, 0, xvp, BF16)
        proj_v(0, xv0, wv_sb, bv_sb, range(4))
        xk1 = load_half(xk, 1, xqk, FP8)
        attn_j(0)
        proj_v(0, xv0, wv_sb, bv_sb, range(4, 8))
        xq1 = load_half(xq, 1, xqk, FP8)
        xv1 = load_half(xv, 1, xvp, BF16)
        wo_sb = load_w(wo, "wo")
        proj_qk(1, xk1, wk_sb, bk_sb, kt)
        attn_j(1, [
            lambda: proj_qk_m(1, xq1, wq_sb, bq_sb, qt, 0),
            lambda: proj_qk_m(1, xq1, wq_sb, bq_sb, qt, 1),
        ] + [(lambda tt=tt: proj_v(1, xv1, wv_sb, bv_sb, [tt]))
             for tt in range(4)])
        proj_v(1, xv1, wv_sb, bv_sb, range(4, 8))
        # exp(j=2,3) runs ~60us on the scalar engine but the PE only owes
        # ~44us of S/PV there -- outproj pieces of finished js fill the gap.
        attn_j(2, op_pieces(0, wo_sb))
        attn_j(3, op_pieces(1, wo_sb) + op_pieces(2, wo_sb))
        for f in op_pieces(3, wo_sb):
            f()


_PROGRAMS = {}


def _build_program(reps=1, loop=0):
    nc = bacc.Bacc("TRN2", target_bir_lowering=False, debug=False,
                   num_devices=NCORES)
    xq = nc.declare_dram_parameter("xq_t", [128, 2, DCH, 1024], FP8, isOutput=False)
    xk = nc.declare_dram_parameter("xk_t", [128, 2, DCH, 1024], FP8, isOutput=False)
    xv = nc.declare_dram_parameter("xv_t", [128, 2, DCH, 1024], BF16, isOutput=False)
    wq = nc.declare_dram_parameter("wq", [128, DCH, GC], FP8, isOutput=False)
    wk = nc.declare_dram_parameter("wk", [128, DCH, GC], FP8, isOutput=False)
    wv = nc.declare_dram_parameter("wv", [128, DCH, GC], BF16, isOutput=False)
    bq = nc.declare_dram_parameter("bq", [GC], F32, isOutput=False)
    bk = nc.declare_dram_parameter("bk", [GC], F32, isOutput=False)
    bv = nc.declare_dram_parameter("bv", [GC], BF16, isOutput=False)
    wo = nc.declare_dram_parameter("wo", [128, GC // 128, D], BF16, isOutput=False)
    out = nc.declare_dram_parameter("out_p", [S, D], BF16, isOutput=True)
    with tile.TileContext(nc) as tc:
        if loop:
            with tc.For_i(0, loop, 1):
                _emit(tc, xq, xk, xv, wq, wk, wv, bq, bk, bv, wo, out)
        else:
            for _ in range(reps):
                _emit(tc, xq, xk, xv, wq, wk, wv, bq, bk, bv, wo, out)
    nc.compile()
    return nc


def get_program(reps=1, loop=0):
    key = (reps, loop)
    if key not in _PROGRAMS:
        _PROGRAMS[key] = _build_program(reps, loop)
    return _PROGRAMS[key]


def make_in_maps(inputs):
    q = np.asarray(inputs["query"], np.float32)
    k = np.asarray(inputs["key"], np.float32)
    v = np.asarray(inputs["value"], np.float32)
    Wq = np.asarray(inputs["Wq"], np.float32)
    Wk = np.asarray(inputs["Wk"], np.float32)
    Wv = np.asarray(inputs["Wv"], np.float32)
    bq = np.asarray(inputs["bq"], np.float32)
    bk = np.asarray(inputs["bk"], np.float32)
    bv = np.asarray(inputs["bv"], np.float32)
    Wo = np.asarray(inputs["Wo"], np.float32)
    def ximg(x, b, dt):
        # [D, S] -> [128, 2 halves, DCH, 1024] SBUF image
        xt = np.ascontiguousarray(x[b].T).astype(dt)
        return np.ascontiguousarray(
            xt.reshape(DCH, 128, 2, 1024).transpose(1, 2, 0, 3))

    def wimg(w, dt):
        # [D or GC, n] -> [128, chunks, n] SBUF image
        nch = w.shape[0] // 128
        return np.ascontiguousarray(
            w.astype(dt).reshape(nch, 128, w.shape[1]).transpose(1, 0, 2))

    xt = [[ximg(x, b, dt) for x, dt in
           ((q, NPFP8), (k, NPFP8), (v, NPBF16))] for b in range(B)]
    in_maps = []
    for core in range(NCORES):
        b, g = divmod(core, NGROUPS)
        cs = slice(g * GC, (g + 1) * GC)
        in_maps.append({
            "xq_t": xt[b][0], "xk_t": xt[b][1], "xv_t": xt[b][2],
            "wq": wimg(Wq[:, cs] * 8.0, NPFP8),
            "wk": wimg(Wk[:, cs] * 8.0, NPFP8),
            "wv": wimg(Wv[:, cs], NPBF16),
            "bq": np.ascontiguousarray(bq[cs] * 8.0),
            "bk": np.ascontiguousarray(bk[cs] * 8.0),
            "bv": np.ascontiguousarray(bv[cs]).astype(NPBF16),
            "wo": wimg(Wo[cs, :], NPBF16),
        })
    return in_maps


def combine_outputs(results, inputs):
    bo = np.asarray(inputs["bo"], np.float32)
    out = np.zeros((B, S, D), np.float32)
    for core in range(NCORES):
        out[core // NGROUPS] += results[core]["out_p"].astype(np.float32)
    out += bo
    return out


def kernel(**inputs):
    nc = get_program()
    in_maps = make_in_maps(inputs)
    res = run_bass_kernel_spmd(nc, in_maps, core_ids=list(range(NCORES)))
    return combine_outputs(res.results, inputs)



# revision 13
# speedup vs baseline: 2.8328x; 2.8328x over previous
"""Distributed causal multi-head attention for one TRN2 chip (8 NeuronCores).

Sharding: 2 batches x 4 head-groups. Core c handles batch c//4 and heads
[4*(c%4), 4*(c%4)+4). Wq/Wk/Wv are column-sliced per head group, Wo is
row-sliced; each core emits a partial (S, D) output and the host sums the
4 partials per batch and adds bo. No on-device collectives.

Per-core dataflow (all matmuls contract over the SBUF partition dim):
  inputs are fed pre-transposed (D, S) so that
    Q^T, K^T = Wq/Wk-chunk.T @ X^T        (heads on partitions)
    V        = X^T-chunk.T  @ Wv          (tokens on partitions, + ones col)
    S^T      = K_h^T-chunk.T @ Q_h^T      (keys on partitions)
    P^T      = exp(scale * S^T)           (causal: restricted N + affine_select)
    O^T      = [V_h | 1].T @ P^T          (row 64 = softmax denominators)
    out      = O^T-chunk.T  @ Wo          (natural layout, DMA out)
"""

import math
import os
import sys

import ml_dtypes
import numpy as np

for _p in ("/opt/trn_rl_repo", "/root/.axon_site/_ro/trn_rl_repo"):
    if os.path.isdir(_p) and _p not in sys.path:
        sys.path.insert(0, _p)

import concourse.bass as bass
import concourse.mybir as mybir
import concourse.tile as tile
from concourse import bacc
from concourse.bass_utils import run_bass_kernel_spmd

F32 = mybir.dt.float32
F32R = mybir.dt.float32r
BF16 = mybir.dt.bfloat16
FP8 = mybir.dt.float8e4
NPBF16 = ml_dtypes.bfloat16
NPFP8 = ml_dtypes.float8_e4m3
AF = mybir.ActivationFunctionType
ALU = mybir.AluOpType


def _r(ap):
    # float32r: same bits as f32, streams the PE at 1 col/cycle (vs 4 for f32)
    return ap.bitcast(F32R)

B, S, D = 2, 2048, 1024
H, HD = 16, 64
NCORES, NGROUPS = 8, 4
GC = D // NGROUPS            # 256 attention columns per core (4 heads)
GH = GC // HD                # heads per core
SCALE = 1.0 / math.sqrt(S)   # reference divides by sqrt(Sk), not sqrt(hd)
WSC = 8.0                    # fp8 pre-scale on Wq/Wk (and bq/bk)
SCALE_QK = SCALE / (WSC * WSC)
DCH = D // 128               # contraction chunks over model dim
ST = S // 128                # 128-row token tiles
NJ = S // 512                # 512-wide query tiles


def _emit(tc, xq, xk, xv, wq, wk, wv, bq, bk, bv, wo, out):
    nc = tc.nc
    from contextlib import ExitStack

    with ExitStack() as ctx:
        const = ctx.enter_context(tc.tile_pool(name="const", bufs=1))
        perm = ctx.enter_context(tc.tile_pool(name="perm", bufs=1))
        xqk = ctx.enter_context(tc.tile_pool(name="xqk", bufs=4))
        xvp = ctx.enter_context(tc.tile_pool(name="xvp", bufs=2))
        ptp = ctx.enter_context(tc.tile_pool(name="ptp", bufs=5))
        nrm = ctx.enter_context(tc.tile_pool(name="nrm", bufs=4))
        outp = ctx.enter_context(tc.tile_pool(name="outp", bufs=1))
        pp = ctx.enter_context(tc.tile_pool(name="pp", bufs=2, space="PSUM"))
        sp = ctx.enter_context(tc.tile_pool(name="sp", bufs=2, space="PSUM"))
        op = ctx.enter_context(tc.tile_pool(name="op", bufs=2, space="PSUM"))

        def load_w(dram, tag):
            nch = dram.shape[0] // 128
            t = const.tile([128, nch, dram.shape[1]], dram.dtype, tag=tag, name=tag)
            for c in range(nch):
                nc.sync.dma_start(out=t[:, c, :], in_=dram[128 * c:128 * (c + 1), :])
            return t

        def load_b(dram, tag):
            t = const.tile([128, 2], F32, tag=tag, name=tag)
            nc.sync.dma_start(out=t[:], in_=dram.ap().rearrange("(m p) -> p m", p=128))
            return t

        # persistent activation tiles --------------------------------------
        qt = [[perm.tile([128, 512], BF16, tag=f"qt{m}_{jj}", name=f"qt{m}_{jj}")
               for jj in range(NJ)] for m in range(2)]
        kt = [[perm.tile([128, 512], BF16, tag=f"kt{m}_{jj}", name=f"kt{m}_{jj}")
               for jj in range(NJ)] for m in range(2)]
        ot = [[perm.tile([128, 512], BF16, tag=f"ot{c}_{j}", name=f"ot{c}_{j}")
               for j in range(NJ)] for c in range(2)]
        ones_f = const.tile([128, GH], BF16, tag="onesf", name="ones_f")
        nc.vector.memset(ones_f[:], 1.0)
        vaug = []
        for t in range(ST):
            vt = perm.tile([128, GH, HD + 1], BF16, tag=f"vaug{t}", name=f"vaug{t}")
            nc.vector.tensor_copy(out=vt[:, :, HD:HD + 1], in_=ones_f[:, :, None])
            vaug.append(vt)
        ones_row = const.tile([1, 128], BF16, tag="ones", name="ones")
        nc.vector.tensor_copy(out=ones_row[:], in_=ones_f[0:1, 0:1].broadcast_to([1, 128]))

        def load_half(x_dram, ha, pool, dt):
            xt = pool.tile([128, DCH, 1024], dt, tag=f"x{ha}", name="xct")
            for d in range(DCH):
                nc.sync.dma_start(
                    out=xt[:, d, :],
                    in_=x_dram[128 * d:128 * (d + 1), 1024 * ha:1024 * (ha + 1)])
            return xt

        def proj_qk_m(ha, xt, w_sb, b_sb, dst, m, act_bias=False):
            ps = [pp.tile([128, 512], F32, tag="pp", name="pqk") for _ in range(2)]
            for d in range(DCH):
                for jj in range(2):
                    nc.tensor.matmul(
                        ps[jj][:],
                        w_sb[:, d, 128 * m:128 * (m + 1)],
                        xt[:, d, 512 * jj:512 * (jj + 1)],
                        start=(d == 0), stop=(d == DCH - 1))
            for jj in range(2):
                if act_bias and jj == 1:
                    # scalar engine is idle here; halves PSUM-evacuation latency
                    nc.scalar.activation(
                        out=dst[m][2 * ha + jj][:], in_=ps[jj][:],
                        func=AF.Identity, bias=b_sb[:, m:m + 1])
                else:
                    nc.vector.tensor_scalar_add(
                        out=dst[m][2 * ha + jj][:], in0=ps[jj][:],
                        scalar1=b_sb[:, m:m + 1])

        def proj_qk(ha, xt, w_sb, b_sb, dst, act_bias=False):
            for m in range(2):
                proj_qk_m(ha, xt, w_sb, b_sb, dst, m, act_bias)

        def proj_v(ha, xt, wv_sb, bv_sb, tts):
            for tt in tts:
                t = 8 * ha + tt
                ps = pp.tile([128, GC], F32, tag="pp", name="pv")
                for d in range(DCH):
                    nc.tensor.matmul(
                        ps[:],
                        xt[:, d, 128 * tt:128 * (tt + 1)],
                        wv_sb[:, d, :],
                        start=(d == 0), stop=False)
                nc.tensor.matmul(
                    ps[:], ones_row[:], bv_sb[:], start=False, stop=True)
                nc.vector.tensor_copy(
                    out=vaug[t][:, :, 0:HD],
                    in_=ps[:].rearrange("p (h e) -> p h e", h=GH))

        def attn_j(j, fillers=()):
            """Pipelined attention for all GH heads of query tile j.

            Emits units (h, chunk-pair): S -> exp -> [mask]; PV trails LAG
            units behind so the PE never sits on the exp latency. `fillers`
            are zero-arg emitters (outproj pieces) spread between units to
            soak up PE idle while the scalar engine chews exp.
            """
            nch = 4 * (j + 1)
            ncp = nch // 2
            units = [(h, cp) for h in range(GH) for cp in range(ncp)]
            LAG = 3
            fillers = list(fillers)
            nf = len(fillers)
            nu = len(units)
            otps = {}
            pend = []

            def emit_S(h, cp):
                c, rr = divmod(h, 2)
                rr *= HD
                sp2 = sp.tile([128, 1024], F32, tag="sp2", name="sp2")
                offs = []
                for k2 in range(2):
                    i = 2 * cp + k2
                    o = i - 4 * j
                    off = min(128 * o, 256) if o > 0 else 0
                    offs.append((i, o, off))
                    nc.tensor.matmul(
                        sp2[:, 512 * k2 + off:512 * (k2 + 1)],
                        kt[c][i // 4][rr:rr + HD, 128 * (i % 4):128 * (i % 4) + 128],
                        qt[c][j][rr:rr + HD, off:512],
                        start=True, stop=True)
                pt2 = ptp.tile([128, 1024], BF16, tag="pt", name="ptt")
                off0 = offs[0][2]
                nc.scalar.activation(
                    out=pt2[:, off0:1024], in_=sp2[:, off0:1024],
                    func=AF.Exp, scale=SCALE_QK)
                for k2, (i, o, off) in enumerate(offs):
                    if o >= 0:
                        w = 256 if o == 3 else 128
                        nc.gpsimd.affine_select(
                            out=pt2[:, 512 * k2 + off:512 * k2 + off + w],
                            in_=pt2[:, 512 * k2 + off:512 * k2 + off + w],
                            compare_op=ALU.is_ge, fill=0.0, base=off - 128 * o,
                            pattern=[[1, w]], channel_multiplier=-1)
                return pt2, offs

            def emit_PV(h, cp, pt2, offs):
                for k2, (i, o, off) in enumerate(offs):
                    nc.tensor.matmul(
                        otps[h][:, off:512], vaug[i][:, h, :],
                        pt2[:, 512 * k2 + off:512 * (k2 + 1)],
                        start=(i == 0), stop=(i == nch - 1),
                        skip_group_check=True)

            def norm(h):
                c, rr = divmod(h, 2)
                rr *= HD
                otp = otps[h]
                rc = nrm.tile([1, 512], F32, tag="rc", name="rc")
                nc.vector.reciprocal(rc[:], otp[HD:HD + 1, :])
                bc = nrm.tile([HD, 512], F32, tag="bc", name="bc")
                nc.gpsimd.partition_broadcast(bc[:], rc[:])
                nc.vector.tensor_tensor(
                    out=ot[c][j][rr:rr + HD, :], in0=otp[0:HD, :], in1=bc[:],
                    op=ALU.mult)

            nxt_fill = 0

            def drain_one():
                h2, cp2, pt2, offs2 = pend.pop(0)
                emit_PV(h2, cp2, pt2, offs2)
                if cp2 == ncp - 1:
                    norm(h2)

            for idx, (h, cp) in enumerate(units):
                if cp == 0:
                    otps[h] = op.tile([HD + 1, 512], F32, tag="op", name="otp")
                pend.append((h, cp) + emit_S(h, cp))
                if len(pend) > LAG:
                    drain_one()
                while nf and nxt_fill < (idx + 1) * nf // nu:
                    fillers[nxt_fill]()
                    nxt_fill += 1
            while pend:
                drain_one()
            while nxt_fill < nf:
                fillers[nxt_fill]()
                nxt_fill += 1

        obs = {}

        def outproj_piece(j, wo_sb, tt):
            if tt == 0:
                obs[j] = outp.tile([128, 4, D], BF16, tag="ob", name="ob")
            ob = obs[j]
            for n2 in range(2):
                fpt = pp.tile([128, 512], F32, tag="pp", name="fpt")
                for c in range(2):
                    nc.tensor.matmul(
                        fpt[:],
                        ot[c][j][:, 128 * tt:128 * (tt + 1)],
                        wo_sb[:, c, 512 * n2:512 * (n2 + 1)],
                        start=(c == 0), stop=(c == 1))
                if n2 == 0:
                    nc.vector.tensor_copy(
                        out=ob[:, tt, 512 * n2:512 * (n2 + 1)], in_=fpt[:])
                else:
                    nc.scalar.copy(out=ob[:, tt, 512 * n2:512 * (n2 + 1)], in_=fpt[:])
            nc.sync.dma_start(
                out=out[128 * (4 * j + tt):128 * (4 * j + tt) + 128, :],
                in_=ob[:, tt, :])

        def op_pieces(j, wo_sb, tts=range(4)):
            return [(lambda j=j, tt=tt: outproj_piece(j, wo_sb, tt))
                    for tt in tts]

        # ---- emission schedule --------------------------------------------
        # Loads are emitted just-in-time (a consumer waits on the DMA-queue
        # counter, so hoisting unrelated loads ahead of it delays its start);
        # PE-heavy projection/outproj pieces are spread as fillers inside the
        # ACT(exp)-bound attention stretches so no engine starves.
        wk_sb = load_w(wk, "wk"); bk_sb = load_b(bk, "bk")
        xk0 = load_half(xk, 0, xqk, FP8)
        proj_qk(0, xk0, wk_sb, bk_sb, kt, act_bias=True)
        wq_sb = load_w(wq, "wq"); bq_sb = load_b(bq, "bq")
        xq0 = load_half(xq, 0, xqk, FP8)
        proj_qk(0, xq0, wq_sb, bq_sb, qt, act_bias=True)
        wv_sb = load_w(wv, "wv")
        bv_sb = const.tile([1, GC], BF16, tag="bv", name="bv")
        nc.sync.dma_start(out=bv_sb[:], in_=bv.ap().rearrange("(o n) -> o n", o=1))
        xv0 = load_half(xv, 0, xvp, BF16)
        proj_v(0, xv0, wv_sb, bv_sb, range(4))
        xk1 = load_half(xk, 1, xqk, FP8)
        attn_j(0)
        proj_v(0, xv0, wv_sb, bv_sb, range(4, 8))
        xq1 = load_half(xq, 1, xqk, FP8)
        xv1 = load_half(xv, 1, xvp, BF16)
        wo_sb = load_w(wo, "wo")
        proj_qk(1, xk1, wk_sb, bk_sb, kt)
        attn_j(1, [
            lambda: proj_qk_m(1, xq1, wq_sb, bq_sb, qt, 0),
            lambda: proj_qk_m(1, xq1, wq_sb, bq_sb, qt, 1),
        ] + [(lambda tt=tt: proj_v(1, xv1, wv_sb, bv_sb, [tt]))
             for tt in range(4)])
        proj_v(1, xv1, wv_sb, bv_sb, range(4, 8))
        # exp(j=2,3) runs ~60us on the scalar engine but the PE only owes
        # ~44us of S/PV there -- outproj pieces of finished js fill the gap.
        attn_j(2, op_pieces(0, wo_sb))
        attn_j(3, op_pieces(1, wo_sb) + op_pieces(2, wo_sb))
        for f in op_pieces(3, wo_sb):
            f()


_PROGRAMS = {}


def _build_program(reps=1, loop=0):
    nc = bacc.Bacc("TRN2", target_bir_lowering=False, debug=False,
                   num_devices=NCORES)
    xq = nc.declare_dram_parameter("xq_t", [D, S], FP8, isOutput=False)
    xk = nc.declare_dram_parameter("xk_t", [D, S], FP8, isOutput=False)
    xv = nc.declare_dram_parameter("xv_t", [D, S], BF16, isOutput=False)
    wq = nc.declare_dram_parameter("wq", [D, GC], FP8, isOutput=False)
    wk = nc.declare_dram_parameter("wk", [D, GC], FP8, isOutput=False)
    wv = nc.declare_dram_parameter("wv", [D, GC], BF16, isOutput=False)
    bq = nc.declare_dram_parameter("bq", [GC], F32, isOutput=False)
    bk = nc.declare_dram_parameter("bk", [GC], F32, isOutput=False)
    bv = nc.declare_dram_parameter("bv", [GC], BF16, isOutput=False)
    wo = nc.declare_dram_parameter("wo", [GC, D], BF16, isOutput=False)
    out = nc.declare_dram_parameter("out_p", [S, D], BF16, isOutput=True)
    with tile.TileContext(nc) as tc:
        if loop:
            with tc.For_i(0, loop, 1):
                _emit(tc, xq, xk, xv, wq, wk, wv, bq, bk, bv, wo, out)
        else:
            for _ in range(reps):
                _emit(tc, xq, xk, xv, wq, wk, wv, bq, bk, bv, wo, out)
    nc.compile()
    return nc


def get_program(reps=1, loop=0):
    key = (reps, loop)
    if key not in _PROGRAMS:
        _PROGRAMS[key] = _build_program(reps, loop)
    return _PROGRAMS[key]


def make_in_maps(inputs):
    q = np.asarray(inputs["query"], np.float32)
    k = np.asarray(inputs["key"], np.float32)
    v = np.asarray(inputs["value"], np.float32)
    Wq = np.asarray(inputs["Wq"], np.float32)
    Wk = np.asarray(inputs["Wk"], np.float32)
    Wv = np.asarray(inputs["Wv"], np.float32)
    bq = np.asarray(inputs["bq"], np.float32)
    bk = np.asarray(inputs["bk"], np.float32)
    bv = np.asarray(inputs["bv"], np.float32)
    Wo = np.asarray(inputs["Wo"], np.float32)
    def wimg(w, dt):
        return np.ascontiguousarray(w).astype(dt)

    xt = [[np.ascontiguousarray(x[b].T).astype(dt) for x, dt in
           ((q, NPFP8), (k, NPFP8), (v, NPBF16))] for b in range(B)]
    in_maps = []
    for core in range(NCORES):
        b, g = divmod(core, NGROUPS)
        cs = slice(g * GC, (g + 1) * GC)
        in_maps.append({
            "xq_t": xt[b][0], "xk_t": xt[b][1], "xv_t": xt[b][2],
            "wq": wimg(Wq[:, cs] * 8.0, NPFP8),
            "wk": wimg(Wk[:, cs] * 8.0, NPFP8),
            "wv": wimg(Wv[:, cs], NPBF16),
            "bq": np.ascontiguousarray(bq[cs] * 8.0),
            "bk": np.ascontiguousarray(bk[cs] * 8.0),
            "bv": np.ascontiguousarray(bv[cs]).astype(NPBF16),
            "wo": wimg(Wo[cs, :], NPBF16),
        })
    return in_maps


def combine_outputs(results, inputs):
    bo = np.asarray(inputs["bo"], np.float32)
    out = np.zeros((B, S, D), np.float32)
    for core in range(NCORES):
        out[core // NGROUPS] += results[core]["out_p"].astype(np.float32)
    out += bo
    return out


def kernel(**inputs):
    nc = get_program()
    in_maps = make_in_maps(inputs)
    res = run_bass_kernel_spmd(nc, in_maps, core_ids=list(range(NCORES)))
    return combine_outputs(res.results, inputs)



# revision 19
# speedup vs baseline: 40.8924x; 14.4352x over previous
"""Distributed causal multi-head attention for one TRN2 chip (8 NeuronCores).

Sharding: 2 batches x 4 head-groups. Core c handles batch c//4 and heads
[4*(c%4), 4*(c%4)+4). Wq/Wk/Wv are column-sliced per head group, Wo is
row-sliced; each core emits a partial (S, D) output (bf16) and the host sums
the 4 partials per batch and adds bo. No on-device collectives.

Dtypes: activations/weights stream as bf16 (halves DMA vs the f32 original);
PSUM accumulates f32; P (exp scores) is bf16.

Per-core dataflow (all matmuls contract over the SBUF partition dim):
  inputs are fed pre-transposed (D, S) so that
    Q^T, K^T = Wq/Wk-chunk.T @ X^T        (heads on partitions)
    V        = X^T-chunk.T  @ Wv          (tokens on partitions, + ones col)
    S^T      = K_h^T-chunk.T @ Q_h^T      (keys on partitions)
    P^T      = exp(scale * S^T)           (causal: restricted N + affine_select)
    O^T      = [V_h | 1].T @ P^T          (row 64 = softmax denominators)
    out      = O^T-chunk.T  @ Wo          (natural layout, DMA out, bf16)

Schedule: attention for each 512-query tile j is emitted as a software
pipeline over (head, key-chunk-pair) units -- scores run LAG=3 units ahead
of PV so the PE never sits on the scalar-engine exp latency; projections for
the second half of the sequence and the output projections of finished j
tiles are spread as PE filler inside the exp-bound attention stretches.
Loads are per-128-row-chunk DMAs emitted just ahead of their consumers
(batching them into big strided DMAs measured slower on HW; so did moving
PSUM evacuation onto the scalar engine, fp8 for the score path, and a
single-buffered 4-bank score PSUM tile -- all A/B'd on hardware and
reverted).
"""

import math
import os
import sys

import ml_dtypes
import numpy as np

for _p in ("/opt/trn_rl_repo", "/root/.axon_site/_ro/trn_rl_repo"):
    if os.path.isdir(_p) and _p not in sys.path:
        sys.path.insert(0, _p)

import concourse.bass as bass
import concourse.mybir as mybir
import concourse.tile as tile
from concourse import bacc
from concourse.bass_utils import run_bass_kernel_spmd

F32 = mybir.dt.float32
F32R = mybir.dt.float32r
BF16 = mybir.dt.bfloat16
FP8 = mybir.dt.float8e4
NPBF16 = ml_dtypes.bfloat16
NPFP8 = ml_dtypes.float8_e4m3
AF = mybir.ActivationFunctionType
ALU = mybir.AluOpType


def _r(ap):
    # float32r: same bits as f32, streams the PE at 1 col/cycle (vs 4 for f32)
    return ap.bitcast(F32R)

B, S, D = 2, 2048, 1024
H, HD = 16, 64
NCORES, NGROUPS = 8, 4
GC = D // NGROUPS            # 256 attention columns per core (4 heads)
GH = GC // HD                # heads per core
SCALE = 1.0 / math.sqrt(S)   # reference divides by sqrt(Sk), not sqrt(hd)
WSC = 8.0                    # fp8 pre-scale on Wq/Wk (and bq/bk)
SCALE_QK = SCALE
DCH = D // 128               # contraction chunks over model dim
ST = S // 128                # 128-row token tiles
NJ = S // 512                # 512-wide query tiles


def _emit(tc, xq, xk, xv, wq, wk, wv, bq, bk, bv, wo, out):
    nc = tc.nc
    from contextlib import ExitStack

    with ExitStack() as ctx:
        const = ctx.enter_context(tc.tile_pool(name="const", bufs=1))
        perm = ctx.enter_context(tc.tile_pool(name="perm", bufs=1))
        xqk = ctx.enter_context(tc.tile_pool(name="xqk", bufs=8))
        xvp = ctx.enter_context(tc.tile_pool(name="xvp", bufs=8))
        ptp = ctx.enter_context(tc.tile_pool(name="ptp", bufs=5))
        nrm = ctx.enter_context(tc.tile_pool(name="nrm", bufs=4))
        outp = ctx.enter_context(tc.tile_pool(name="outp", bufs=1))
        pp = ctx.enter_context(tc.tile_pool(name="pp", bufs=2, space="PSUM"))
        sp = ctx.enter_context(tc.tile_pool(name="sp", bufs=2, space="PSUM"))
        op = ctx.enter_context(tc.tile_pool(name="op", bufs=2, space="PSUM"))

        def load_w(dram, tag):
            nch = dram.shape[0] // 128
            t = const.tile([128, nch, dram.shape[1]], dram.dtype, tag=tag, name=tag)
            for c in range(nch):
                nc.sync.dma_start(out=t[:, c, :], in_=dram[128 * c:128 * (c + 1), :])
            return t

        def load_b(dram, tag):
            t = const.tile([128, 2], F32, tag=tag, name=tag)
            nc.sync.dma_start(out=t[:], in_=dram.ap().rearrange("(m p) -> p m", p=128))
            return t

        # persistent activation tiles --------------------------------------
        qt = [[perm.tile([128, 512], BF16, tag=f"qt{m}_{jj}", name=f"qt{m}_{jj}")
               for jj in range(NJ)] for m in range(2)]
        kt = [[perm.tile([128, 512], BF16, tag=f"kt{m}_{jj}", name=f"kt{m}_{jj}")
               for jj in range(NJ)] for m in range(2)]
        ot = [[perm.tile([128, 512], BF16, tag=f"ot{c}_{j}", name=f"ot{c}_{j}")
               for j in range(NJ)] for c in range(2)]
        ones_f = const.tile([128, GH], BF16, tag="onesf", name="ones_f")
        nc.vector.memset(ones_f[:], 1.0)
        vaug = []
        for t in range(ST):
            vt = perm.tile([128, GH, HD + 1], BF16, tag=f"vaug{t}", name=f"vaug{t}")
            nc.vector.tensor_copy(out=vt[:, :, HD:HD + 1], in_=ones_f[:, :, None])
            vaug.append(vt)
        ones_row = const.tile([1, 128], BF16, tag="ones", name="ones")
        nc.vector.tensor_copy(out=ones_row[:], in_=ones_f[0:1, 0:1].broadcast_to([1, 128]))

        def load_half(x_dram, ha, pool, dt, tag):
            tiles = []
            for d in range(DCH):
                xt = pool.tile([128, 1024], dt, tag=f"{tag}{ha}", name="xct")
                nc.sync.dma_start(
                    out=xt[:],
                    in_=x_dram[128 * d:128 * (d + 1), 1024 * ha:1024 * (ha + 1)])
                tiles.append(xt)
            return tiles

        def proj_qk_m(ha, xt, w_sb, b_sb, dst, m, act_bias=False):
            ps = [pp.tile([128, 512], F32, tag="pp", name="pqk") for _ in range(2)]
            for d in range(DCH):
                for jj in range(2):
                    nc.tensor.matmul(
                        ps[jj][:],
                        w_sb[:, d, 128 * m:128 * (m + 1)],
                        xt[d][:, 512 * jj:512 * (jj + 1)],
                        start=(d == 0), stop=(d == DCH - 1))
            for jj in range(2):
                nc.vector.tensor_scalar_add(
                    out=dst[m][2 * ha + jj][:], in0=ps[jj][:],
                    scalar1=b_sb[:, m:m + 1])

        def proj_qk(ha, xt, w_sb, b_sb, dst, act_bias=False):
            for m in range(2):
                proj_qk_m(ha, xt, w_sb, b_sb, dst, m, act_bias)

        def proj_v(ha, xt, wv_sb, bv_sb, tts):
            for tt in tts:
                t = 8 * ha + tt
                ps = pp.tile([128, GC], F32, tag="pp", name="pv")
                for d in range(DCH):
                    nc.tensor.matmul(
                        ps[:],
                        xt[d][:, 128 * tt:128 * (tt + 1)],
                        wv_sb[:, d, :],
                        start=(d == 0), stop=False)
                nc.tensor.matmul(
                    ps[:], ones_row[:], bv_sb[:], start=False, stop=True)
                nc.vector.tensor_copy(
                    out=vaug[t][:, :, 0:HD],
                    in_=ps[:].rearrange("p (h e) -> p h e", h=GH))

        def attn_j(j, fillers=()):
            """Pipelined attention for all GH heads of query tile j.

            Emits units (h, chunk-pair): S -> exp -> [mask]; PV trails LAG
            units behind so the PE never sits on the exp latency. `fillers`
            are zero-arg emitters (outproj pieces) spread between units to
            soak up PE idle while the scalar engine chews exp.
            """
            nch = 4 * (j + 1)
            ncp = nch // 2
            units = [(h, cp) for h in range(GH) for cp in range(ncp)]
            LAG = 3
            fillers = list(fillers)
            nf = len(fillers)
            nu = len(units)
            otps = {}
            pend = []

            def emit_S(h, cp):
                c, rr = divmod(h, 2)
                rr *= HD
                sp2 = sp.tile([128, 1024], F32, tag="sp2", name="sp2")
                offs = []
                for k2 in range(2):
                    i = 2 * cp + k2
                    o = i - 4 * j
                    off = min(128 * o, 256) if o > 0 else 0
                    offs.append((i, o, off))
                    nc.tensor.matmul(
                        sp2[:, 512 * k2 + off:512 * (k2 + 1)],
                        kt[c][i // 4][rr:rr + HD, 128 * (i % 4):128 * (i % 4) + 128],
                        qt[c][j][rr:rr + HD, off:512],
                        start=True, stop=True)
                pt2 = ptp.tile([128, 1024], BF16, tag="pt", name="ptt")
                off0 = offs[0][2]
                nc.scalar.activation(
                    out=pt2[:, off0:1024], in_=sp2[:, off0:1024],
                    func=AF.Exp, scale=SCALE_QK)
                for k2, (i, o, off) in enumerate(offs):
                    if o >= 0:
                        w = 256 if o == 3 else 128
                        nc.gpsimd.affine_select(
                            out=pt2[:, 512 * k2 + off:512 * k2 + off + w],
                            in_=pt2[:, 512 * k2 + off:512 * k2 + off + w],
                            compare_op=ALU.is_ge, fill=0.0, base=off - 128 * o,
                            pattern=[[1, w]], channel_multiplier=-1)
                return pt2, offs

            def emit_PV(h, cp, pt2, offs):
                for k2, (i, o, off) in enumerate(offs):
                    nc.tensor.matmul(
                        otps[h][:, off:512], vaug[i][:, h, :],
                        pt2[:, 512 * k2 + off:512 * (k2 + 1)],
                        start=(i == 0), stop=(i == nch - 1),
                        skip_group_check=True)

            def norm(h):
                c, rr = divmod(h, 2)
                rr *= HD
                otp = otps[h]
                rc = nrm.tile([1, 512], F32, tag="rc", name="rc")
                nc.vector.reciprocal(rc[:], otp[HD:HD + 1, :])
                bc = nrm.tile([HD, 512], F32, tag="bc", name="bc")
                nc.gpsimd.partition_broadcast(bc[:], rc[:])
                nc.vector.tensor_tensor(
                    out=ot[c][j][rr:rr + HD, :], in0=otp[0:HD, :], in1=bc[:],
                    op=ALU.mult)

            nxt_fill = 0

            def drain_one():
                h2, cp2, pt2, offs2 = pend.pop(0)
                emit_PV(h2, cp2, pt2, offs2)
                if cp2 == ncp - 1:
                    norm(h2)

            for idx, (h, cp) in enumerate(units):
                if cp == 0:
                    otps[h] = op.tile([HD + 1, 512], F32, tag="op", name="otp")
                pend.append((h, cp) + emit_S(h, cp))
                if len(pend) > LAG:
                    drain_one()
                while nf and nxt_fill < (idx + 1) * nf // nu:
                    fillers[nxt_fill]()
                    nxt_fill += 1
            while pend:
                drain_one()
            while nxt_fill < nf:
                fillers[nxt_fill]()
                nxt_fill += 1

        obs = {}

        def outproj_piece(j, wo_sb, tt):
            if tt == 0:
                obs[j] = outp.tile([128, 4, D], BF16, tag="ob", name="ob")
            ob = obs[j]
            for n2 in range(2):
                fpt = pp.tile([128, 512], F32, tag="pp", name="fpt")
                for c in range(2):
                    nc.tensor.matmul(
                        fpt[:],
                        ot[c][j][:, 128 * tt:128 * (tt + 1)],
                        wo_sb[:, c, 512 * n2:512 * (n2 + 1)],
                        start=(c == 0), stop=(c == 1))
                nc.vector.tensor_copy(
                    out=ob[:, tt, 512 * n2:512 * (n2 + 1)], in_=fpt[:])
            nc.sync.dma_start(
                out=out[128 * (4 * j + tt):128 * (4 * j + tt) + 128, :],
                in_=ob[:, tt, :])

        def op_pieces(j, wo_sb, tts=range(4)):
            return [(lambda j=j, tt=tt: outproj_piece(j, wo_sb, tt))
                    for tt in tts]

        # ---- emission schedule --------------------------------------------
        # Loads are emitted just-in-time (a consumer waits on the DMA-queue
        # counter, so hoisting unrelated loads ahead of it delays its start);
        # PE-heavy projection/outproj pieces are spread as fillers inside the
        # ACT(exp)-bound attention stretches so no engine starves.
        wk_sb = load_w(wk, "wk"); bk_sb = load_b(bk, "bk")
        xk0 = load_half(xk, 0, xqk, BF16, "k")
        proj_qk(0, xk0, wk_sb, bk_sb, kt, act_bias=True)
        wq_sb = load_w(wq, "wq"); bq_sb = load_b(bq, "bq")
        xq0 = load_half(xq, 0, xqk, BF16, "q")
        proj_qk(0, xq0, wq_sb, bq_sb, qt, act_bias=True)
        wv_sb = load_w(wv, "wv")
        bv_sb = const.tile([1, GC], BF16, tag="bv", name="bv")
        nc.sync.dma_start(out=bv_sb[:], in_=bv.ap().rearrange("(o n) -> o n", o=1))
        xv0 = load_half(xv, 0, xvp, BF16, "v")
        proj_v(0, xv0, wv_sb, bv_sb, range(4))
        xk1 = load_half(xk, 1, xqk, BF16, "k")
        attn_j(0)
        proj_v(0, xv0, wv_sb, bv_sb, range(4, 8))
        xq1 = load_half(xq, 1, xqk, BF16, "q")
        xv1 = load_half(xv, 1, xvp, BF16, "v")
        wo_sb = load_w(wo, "wo")
        proj_qk(1, xk1, wk_sb, bk_sb, kt)
        attn_j(1, [
            lambda: proj_qk_m(1, xq1, wq_sb, bq_sb, qt, 0),
            lambda: proj_qk_m(1, xq1, wq_sb, bq_sb, qt, 1),
        ] + [(lambda tt=tt: proj_v(1, xv1, wv_sb, bv_sb, [tt]))
             for tt in range(4)])
        proj_v(1, xv1, wv_sb, bv_sb, range(4, 8))
        # exp(j=2,3) runs ~60us on the scalar engine but the PE only owes
        # ~44us of S/PV there -- outproj pieces of finished js fill the gap.
        attn_j(2, op_pieces(0, wo_sb))
        attn_j(3, op_pieces(1, wo_sb) + op_pieces(2, wo_sb))
        for f in op_pieces(3, wo_sb):
            f()


_PROGRAMS = {}


def _build_program(reps=1, loop=0):
    nc = bacc.Bacc("TRN2", target_bir_lowering=False, debug=False,
                   num_devices=NCORES)
    xq = nc.declare_dram_parameter("xq_t", [D, S], BF16, isOutput=False)
    xk = nc.declare_dram_parameter("xk_t", [D, S], BF16, isOutput=False)
    xv = nc.declare_dram_parameter("xv_t", [D, S], BF16, isOutput=False)
    wq = nc.declare_dram_parameter("wq", [D, GC], BF16, isOutput=False)
    wk = nc.declare_dram_parameter("wk", [D, GC], BF16, isOutput=False)
    wv = nc.declare_dram_parameter("wv", [D, GC], BF16, isOutput=False)
    bq = nc.declare_dram_parameter("bq", [GC], F32, isOutput=False)
    bk = nc.declare_dram_parameter("bk", [GC], F32, isOutput=False)
    bv = nc.declare_dram_parameter("bv", [GC], BF16, isOutput=False)
    wo = nc.declare_dram_parameter("wo", [GC, D], BF16, isOutput=False)
    out = nc.declare_dram_parameter("out_p", [S, D], BF16, isOutput=True)
    with tile.TileContext(nc) as tc:
        if loop:
            with tc.For_i(0, loop, 1):
                _emit(tc, xq, xk, xv, wq, wk, wv, bq, bk, bv, wo, out)
        else:
            for _ in range(reps):
                _emit(tc, xq, xk, xv, wq, wk, wv, bq, bk, bv, wo, out)
    nc.compile()
    return nc


def get_program(reps=1, loop=0):
    key = (reps, loop)
    if key not in _PROGRAMS:
        _PROGRAMS[key] = _build_program(reps, loop)
    return _PROGRAMS[key]


def make_in_maps(inputs):
    q = np.asarray(inputs["query"], np.float32)
    k = np.asarray(inputs["key"], np.float32)
    v = np.asarray(inputs["value"], np.float32)
    Wq = np.asarray(inputs["Wq"], np.float32)
    Wk = np.asarray(inputs["Wk"], np.float32)
    Wv = np.asarray(inputs["Wv"], np.float32)
    bq = np.asarray(inputs["bq"], np.float32)
    bk = np.asarray(inputs["bk"], np.float32)
    bv = np.asarray(inputs["bv"], np.float32)
    Wo = np.asarray(inputs["Wo"], np.float32)
    def wimg(w, dt):
        return np.ascontiguousarray(w).astype(dt)

    xt = [[np.ascontiguousarray(x[b].T).astype(dt) for x, dt in
           ((q, NPBF16), (k, NPBF16), (v, NPBF16))] for b in range(B)]
    in_maps = []
    for core in range(NCORES):
        b, g = divmod(core, NGROUPS)
        cs = slice(g * GC, (g + 1) * GC)
        in_maps.append({
            "xq_t": xt[b][0], "xk_t": xt[b][1], "xv_t": xt[b][2],
            "wq": wimg(Wq[:, cs], NPBF16),
            "wk": wimg(Wk[:, cs], NPBF16),
            "wv": wimg(Wv[:, cs], NPBF16),
            "bq": np.ascontiguousarray(bq[cs]),
            "bk": np.ascontiguousarray(bk[cs]),
            "bv": np.ascontiguousarray(bv[cs]).astype(NPBF16),
            "wo": wimg(Wo[cs, :], NPBF16),
        })
    return in_maps


def combine_outputs(results, inputs):
    bo = np.asarray(inputs["bo"], np.float32)
    out = np.zeros((B, S, D), np.float32)
    for core in range(NCORES):
        out[core // NGROUPS] += results[core]["out_p"].astype(np.float32)
    out += bo
    return out


def kernel(**inputs):
    nc = get_program()
    in_maps = make_in_maps(inputs)
    res = run_bass_kernel_spmd(nc, in_maps, core_ids=list(range(NCORES)))
    return combine_outputs(res.results, inputs)

